# revision 1
# baseline (speedup 1.0000x reference)
"""CRF loss (forward-algorithm log-partition + gold-path score) on 8 Trainium2
NeuronCores.

Algorithm (per batch row):
  log_den = logsumexp over tag paths (forward recursion over S=512 steps)
  log_num = score of the gold tag path
  loss    = mean_b(log_den - log_num)

Device mapping:
  * Linear-space forward recursion:  q_{t+1} = (E_bd @ q_t) * exp(e_t - c0)
    with E_bd = block-diag(exp(transition)), c0 = 6*ln2 a constant rescale
    that keeps q inside fp32/bf16 exponent range.  At the end
    log_den = ln(sum_j q) + S*c0.  This is the only serial part: 511 rounds
    of one 128x128 block-diag matmul (PE) + one PSUM*SBUF multiply (DVE)
    per 128-row chain, two independent chains pipelined per core.
  * Gold score is computed in bulk (no serial chain, no gathers):
      gold_total = sum_{s,j} hp[j,s] * (e[j,s] + V[j,s-1]),
      V = blockdiag(trans) ^T-applied to the one-hot stream:
      V[:, s] = trans[tag_s, :]  via matmuls with the one-hot tensor.
    Products+reductions run as a few large fused ops (PE matmuls, GPSIMD
    adds, DVE tensor_tensor_reduce) fully overlapped with the recursion.
  * Data-parallel over batch: each core takes 256 rows as 2 chains of 128
    rows; each chain packs 4 groups of 32 rows into the 128 partitions with
    the (padded) 32-wide tag dim per group, so the 24x24 tag contraction is
    a single 128x128 block-diagonal matmul per step per chain.

Host side only reshapes/pads/one-hot-encodes inputs into the packed
[group*32+tag, step*32+row] layout; all arithmetic of the loss runs on
device.
"""

import math
import os

import numpy as np
import ml_dtypes

import concourse.bass as bass
import concourse.bacc as bacc
import concourse.tile as tile
import concourse.mybir as mybir
import concourse.bass_utils as bass_utils
from concourse.bass_utils import run_bass_kernel_spmd

BF16 = mybir.dt.bfloat16
F32 = mybir.dt.float32
AF = mybir.ActivationFunctionType
ALU = mybir.AluOpType
NPBF16 = ml_dtypes.bfloat16

B, S, NT = 2048, 512, 24
JP = 32                    # padded tag dim (multiple of 32)
NCORES = 8
RPC = B // NCORES          # rows per core (256)
NCHAIN = 2                 # chains per core, 128 rows each
CS = 64                    # steps per prep chunk
C0 = 6.0 * math.log(2.0)   # per-step rescale of the partition chain
NEG = -30000.0             # pad value; exp -> 0
VMM_FD = 256               # free-dim per gold V matmul
VMM_PER_CHUNK = CS * JP // VMM_FD  # 4


def build_program(s_total=S):
    assert s_total % CS == 0
    nch = s_total // CS
    nc = bacc.Bacc(trn_type="TRN2")
    ep_d = nc.dram_tensor("ep", [RPC, s_total * JP], BF16, kind="ExternalInput")
    hp_d = nc.dram_tensor("hp", [RPC, s_total * JP], BF16, kind="ExternalInput")
    tr_d = nc.dram_tensor("tr", [NT, NT], F32, kind="ExternalInput")
    out_d = nc.dram_tensor("out", [1, 1], F32, kind="ExternalOutput")

    with tile.TileContext(nc) as tc:
        with tc.tile_pool(name="const", bufs=1) as const, \
             tc.tile_pool(name="chunks", bufs=3) as chunks, \
             tc.tile_pool(name="state", bufs=1) as state, \
             tc.tile_pool(name="small", bufs=4) as small, \
             tc.tile_pool(name="pmm", bufs=2, space="PSUM") as pmm, \
             tc.tile_pool(name="pv", bufs=3, space="PSUM") as pv, \
             tc.tile_pool(name="psg", bufs=1, space="PSUM") as psg:

            # ---- constants (memset-only parts; DMA parts deferred) ----
            ebd = const.tile([128, 128], BF16)
            nc.vector.memset(ebd[:, :], 0.0)
            tbd = const.tile([128, 128], BF16)
            nc.vector.memset(tbd[:, :], 0.0)
            osel = const.tile([128, 4], BF16)
            nc.vector.memset(osel[:, :], 0.0)
            for g in range(4):
                nc.vector.memset(osel[32 * g:32 * g + 32, g:g + 1], 1.0)
            ones4 = const.tile([4, 1], F32)
            nc.vector.memset(ones4[:, :], 1.0)
            onesn128 = const.tile([128, 1], F32)
            nc.vector.memset(onesn128[:, :], -1.0)
            negc0 = const.tile([128, 1], F32)
            nc.vector.memset(negc0[:, :], -C0)

            # ---- persistent per-chain state ----
            q = [state.tile([128, JP], BF16, name=f"q{c}")
                 for c in range(NCHAIN)]
            gparts = []
            for c in range(NCHAIN):
                g_t = state.tile([128, 2 * nch], F32, name=f"gparts{c}")
                nc.vector.memset(g_t[:, :], 0.0)
                gparts.append(g_t)

            ep_tiles = [[None] * nch for _ in range(NCHAIN)]
            a1_tiles = [dict() for _ in range(NCHAIN)]
            prod_tiles = [dict() for _ in range(NCHAIN)]
            hp_tiles = [[None] * nch for _ in range(NCHAIN)]
            fh_tiles = [[None] * nch for _ in range(NCHAIN)]
            v_tiles = [[None] * nch for _ in range(NCHAIN)]

            def prep_chunk(c, k):
                ep_t = chunks.tile([128, CS, JP], BF16, tag=f"ep{c}",
                                   name=f"ep{c}_{k}")
                hp_t = chunks.tile([128, CS, JP], BF16, tag=f"hp{c}",
                                   name=f"hp{c}_{k}")
                lo = k * CS * JP
                nc.sync.dma_start(
                    out=ep_t[:, :, :],
                    in_=ep_d[c * 128:(c + 1) * 128, lo:lo + CS * JP]
                    .rearrange("p (s j) -> p s j", j=JP))
                nc.sync.dma_start(
                    out=hp_t[:, :, :],
                    in_=hp_d[c * 128:(c + 1) * 128, lo:lo + CS * JP]
                    .rearrange("p (s j) -> p s j", j=JP))
                fh = chunks.tile([128, CS, JP], BF16, tag=f"fh{c}",
                                 name=f"fh{c}_{k}")
                # F_hat = exp(ep - c0); split so early steps unblock sooner
                for e4 in range(4):
                    ssl = slice(e4 * CS // 4, (e4 + 1) * CS // 4)
                    nc.scalar.activation(fh[:, ssl, :], ep_t[:, ssl, :],
                                         AF.Exp, bias=negc0[:, :])
                ep_tiles[c][k] = ep_t
                hp_tiles[c][k] = hp_t
                fh_tiles[c][k] = fh

            # ---- gold bulk ops (emitted sprinkled through chunk k's steps) --
            def get_vtile(c, k):
                if v_tiles[c][k] is None:
                    v_tiles[c][k] = chunks.tile(
                        [128, (CS + 1) * JP], BF16, tag=f"v{c}",
                        name=f"v{c}_{k}")
                return v_tiles[c][k]

            def gold_vmm(c, k, i):
                # V[:, JP + i*FD : JP + (i+1)*FD] = tbd.T @ hp-slice
                get_vtile(c, k)
                vp = pv.tile([128, VMM_FD], F32, tag="vp", name=f"vp{c}_{k}_{i}")
                hp_t = hp_tiles[c][k]
                nc.tensor.matmul(
                    vp[:, :], tbd[:, :],
                    hp_t[:, :, :].rearrange("p s j -> p (s j)")
                    [:, i * VMM_FD:(i + 1) * VMM_FD],
                    start=True, stop=True)
                nc.scalar.copy(
                    v_tiles[c][k][:, JP + i * VMM_FD: JP + (i + 1) * VMM_FD],
                    vp[:, :])

            def gold_carry(c, k):
                v_t = get_vtile(c, k)
                if k == 0:
                    nc.vector.memset(v_t[:, 0:JP], 0.0)
                else:
                    nc.vector.tensor_copy(v_t[:, 0:JP],
                                          v_tiles[c][k - 1][:, CS * JP:(CS + 1) * JP])

            GH = CS * JP // 2  # 1024-col half-chunk slices

            def gold_add(c, k, h):
                # A1 = ep + V_backshift (DVE, sliced to ride round slack)
                if k not in a1_tiles[c]:
                    a1_tiles[c][k] = chunks.tile([128, CS * JP], BF16,
                                                 tag="a1", name=f"a1_{c}_{k}")
                sel = slice(h * GH, (h + 1) * GH)
                nc.vector.tensor_add(
                    a1_tiles[c][k][:, sel],
                    ep_tiles[c][k][:, :, :].rearrange("p s j -> p (s j)")[:, sel],
                    v_tiles[c][k][:, 0:CS * JP][:, sel])

            def gold_mul(c, k, h):
                if k not in prod_tiles[c]:
                    prod_tiles[c][k] = chunks.tile([128, CS * JP], BF16,
                                                   tag="prod",
                                                   name=f"prod_{c}_{k}")
                sel = slice(h * GH, (h + 1) * GH)
                hp_flat = hp_tiles[c][k][:, :, :].rearrange("p s j -> p (s j)")
                nc.vector.tensor_mul(prod_tiles[c][k][:, sel],
                                     a1_tiles[c][k][:, sel], hp_flat[:, sel])
                # reduce on ScalarE (keeps the DVE recursion bubble-free)
                scr = chunks.tile([128, GH], BF16, tag="scr",
                                  name=f"scr_{c}_{k}_{h}")
                nc.scalar.activation(scr[:, :], prod_tiles[c][k][:, sel],
                                     AF.Copy,
                                     accum_out=gparts[c][:, 2 * k + h:
                                                         2 * k + h + 1])

            # ---- pipeline ----
            for c in range(NCHAIN):
                prep_chunk(c, 0)
            if nch > 1:
                for c in range(NCHAIN):
                    prep_chunk(c, 1)
            # deferred constant builds (their small DMAs queue after the
            # big chunk-0 loads that gate the first recursion rounds)
            tr32 = const.tile([NT, NT], F32)
            nc.sync.dma_start(out=tr32[:, :], in_=tr_d[:, :])
            e24 = const.tile([NT, NT], BF16)
            nc.scalar.activation(e24[:, :], tr32[:, :], AF.Exp)
            t24 = const.tile([NT, NT], BF16)
            nc.vector.tensor_copy(t24[:, :], tr32[:, :])
            for g in range(4):
                # SBUF->SBUF DMA to place blocks on the diagonal
                nc.sync.dma_start(
                    out=ebd[32 * g:32 * g + NT, 32 * g:32 * g + NT],
                    in_=e24[:, :])
                nc.sync.dma_start(
                    out=tbd[32 * g:32 * g + NT, 32 * g:32 * g + NT],
                    in_=t24[:, :])
            for c in range(NCHAIN):
                nc.vector.tensor_copy(q[c][:, :], fh_tiles[c][0][:, 0, :])

            for k in range(nch):
                if k + 2 < nch:
                    for c in range(NCHAIN):
                        prep_chunk(c, k + 2)
                s_lo = 1 if k == 0 else 0
                for sl in range(s_lo, CS):
                    for c in range(NCHAIN):
                        p_t = pmm.tile([128, JP], F32, tag=f"p{c}",
                                       name=f"p{c}_{k}_{sl}")
                        nc.tensor.matmul(p_t[:, :], ebd[:, :], q[c][:, :],
                                         start=True, stop=True)
                        nc.vector.tensor_mul(q[c][:, :], p_t[:, :],
                                             fh_tiles[c][k][:, sl, :])
                    # sprinkle the bulk gold work between recursion rounds,
                    # staggered per chain to avoid paired stalls
                    if sl == 2:
                        for c in range(NCHAIN):
                            gold_carry(c, k)
                    for c in range(NCHAIN):
                        base = 3 + 2 * c
                        if (sl - base) % 4 == 0:
                            i = (sl - base) // 4
                            if 0 <= i < VMM_PER_CHUNK:
                                gold_vmm(c, k, i)
                    if sl == 25:
                        gold_add(c=0, k=k, h=0)
                    elif sl == 28:
                        gold_add(c=1, k=k, h=0)
                    elif sl == 41:
                        gold_add(c=0, k=k, h=1)
                    elif sl == 44:
                        gold_add(c=1, k=k, h=1)
                    elif sl == 33:
                        gold_mul(c=0, k=k, h=0)
                    elif sl == 36:
                        gold_mul(c=1, k=k, h=0)
                    elif sl == 49:
                        gold_mul(c=0, k=k, h=1)
                    elif sl == 52:
                        gold_mul(c=1, k=k, h=1)

            # ---- finalization ----
            ldr = []
            for c in range(NCHAIN):
                tq = psg.tile([4, JP], F32, tag="tg", name=f"tq{c}")
                nc.tensor.matmul(tq[:, :], osel[:, :], q[c][:, :],
                                 start=True, stop=True)
                ld = small.tile([4, JP], F32, tag="ld", name=f"ld{c}")
                nc.scalar.activation(ld[:, :], tq[:, :], AF.Ln)
                r_t = small.tile([4, 1], F32, tag="rs", name=f"rs{c}")
                nc.vector.reduce_sum(r_t[:, :], ld[:, :],
                                     axis=mybir.AxisListType.X)
                ldr.append(r_t)
            lsum = small.tile([4, 1], F32, tag="lsum", name="lsum")
            nc.vector.tensor_add(lsum[:, :], ldr[0][:, :], ldr[1][:, :])

            gr = []
            for c in range(NCHAIN):
                g_t = small.tile([128, 1], F32, tag="gr", name=f"gr{c}")
                nc.vector.reduce_sum(g_t[:, :], gparts[c][:, :],
                                     axis=mybir.AxisListType.X)
                gr.append(g_t)
            gsum = small.tile([128, 1], F32, tag="gsum", name="gsum")
            nc.vector.tensor_add(gsum[:, :], gr[0][:, :], gr[1][:, :])

            finl = psg.tile([1, 1], F32, tag="tg", name="finl")
            nc.tensor.matmul(finl[:, :], ones4[:, :], lsum[:, :],
                             start=True, stop=True)
            fing = psg.tile([1, 1], F32, tag="tg", name="fing")
            nc.tensor.matmul(fing[:, :], onesn128[:, :], gsum[:, :],
                             start=True, stop=True)
            sl_t = small.tile([1, 1], F32, tag="outv", name="sl_t")
            nc.scalar.copy(sl_t[:, :], finl[:, :])
            sg_t = small.tile([1, 1], F32, tag="outv", name="sg_t")
            nc.scalar.copy(sg_t[:, :], fing[:, :])
            outv = small.tile([1, 1], F32, tag="outv", name="outv")
            # + per-core constant: RPC rows * S steps * c0  (sg holds -gold)
            nc.vector.scalar_tensor_tensor(
                outv[:, :], sl_t[:, :], float(RPC) * float(s_total) * C0,
                sg_t[:, :], ALU.add, ALU.add)
            nc.sync.dma_start(out=out_d[:, :], in_=outv[:, :])
    _bacc_compile_no_ldw_split(nc)
    return nc


def _bacc_compile_no_ldw_split(nc):
    """Bacc.compile() minus move_matmul_waits_to_ldweights (so our
    Ldweights-dedup below stays valid; generate_event_semaphores handles
    multi-wait matmuls)."""
    from concourse import inst_simplify

    nc.insert_bir_kernel_barrier_sem_inc()
    nc.generate_event_semaphores()
    nc.remove_dead_instructions_after_branch()
    nc.validate_blocks()
    nc.dce_regs()
    nc.thread_jumps()
    nc.remove_dead_blocks()
    nc.remove_dead_allocations()
    nc.verify_switch_hints()
    nc.alloc_regs()
    inst_simplify.simplify(nc)
    nc.fuse_regops()
    nc.fuse_blocks()
    nc.replace_nops_with_events()
    for engine in nc.engines:
        nc.fuse_nops(engine)
    nc.remove_dead_nops()
    nc.remove_dangling_data()
    nc.generate_event_semaphores()
    nc.insert_library_loads()
    nc.insert_act_table_loads()
    nc.insert_hostgen_rebases()
    nc.codegen_inst_isa_subclasses()
    _dedup_ldweights(nc)


def _dedup_ldweights(nc):
    """Drop PE Ldweights that reload the already-loaded stationary weights.

    codegen_inst_isa_subclasses splits every matmul into Ldweights+Matmult;
    long runs of recursion matmuls share one stationary matrix, so the
    repeated 128-row reload (~100ns each) would clog the PE stream.  Keep
    any Ldweights carrying sync waits/updates to preserve semaphores."""
    removed = 0
    for fn in nc.m.functions:
        for blk in fn.blocks:
            cur_sig = None
            out = []
            for inst in blk.instructions:
                tname = type(inst).__name__
                if tname == "InstLdweights":
                    sig = inst.concise().split("Ldweights", 1)[-1]
                    if sig == cur_sig and not inst.has_wait() \
                            and not inst.has_update():
                        removed += 1
                        continue
                    cur_sig = sig
                elif tname in ("InstMatmult", "InstMatmultMx"):
                    pass  # uses loaded weights; state unchanged
                elif str(inst.engine) == "EngineType.PE" and tname not in (
                        "InstEventSemaphore", "InstNop", "InstSemWait"):
                    cur_sig = None
                out.append(inst)
            blk.instructions = out
    return removed


def prep_inputs(emissions, tags, s_total=S):
    """Host-side layout prep: per-core packed-transposed bf16 tensors.

    ep[core][chain*128 + G*32 + j, s*32 + b] = emissions[row, s, j] (pad NEG)
    hp likewise one-hot(tags) in {0, 1}.
    row = core*256 + chain*128 + G*32 + b.
    """
    e = np.asarray(emissions)[:, :s_total, :]
    t = np.asarray(tags)[:, :s_total]
    eb = e.astype(NPBF16)
    ep = np.full((B, s_total, JP), NEG, dtype=NPBF16)
    ep[:, :, :NT] = eb
    hp = np.zeros((B, s_total, JP), dtype=NPBF16)
    np.put_along_axis(hp, t[..., None], np.asarray(1.0, NPBF16), axis=2)

    def pack(x):
        # [B, s, JP] -> [cores, chain, G, b, s, j] -> [cores, chain, G, j, s, b]
        x = x.reshape(NCORES, NCHAIN, 4, 32, s_total, JP)
        x = np.ascontiguousarray(x.transpose(0, 1, 2, 5, 4, 3))
        return x.reshape(NCORES, RPC, s_total * JP)

    return pack(ep), pack(hp)


_PROGRAM_CACHE = {}


def kernel(emissions, tags, mask, transition_scores):
    assert np.asarray(mask).min() == 1, "kernel assumes all-ones mask"
    ep, hp = prep_inputs(emissions, tags)
    tr = np.ascontiguousarray(np.asarray(transition_scores, dtype=np.float32))

    if S not in _PROGRAM_CACHE:
        _PROGRAM_CACHE[S] = build_program(S)
    nc = _PROGRAM_CACHE[S]

    in_maps = [
        {"ep": np.ascontiguousarray(ep[c]), "hp": np.ascontiguousarray(hp[c]),
         "tr": tr}
        for c in range(NCORES)
    ]
    res = run_bass_kernel_spmd(nc, in_maps, core_ids=list(range(NCORES)))
    partials = [float(r["out"][0, 0]) for r in res.results]
    return np.float32(sum(partials) / B)

